# revision 1
# baseline (speedup 1.0000x reference)
"""Trainium2 Bass kernel for nn_MoEBottleneck (moe_routing).

Data-parallel over batch: 64 samples sharded 8-per-core across 8 NeuronCores.
Per core, samples are processed in pairs packed onto the 128 SBUF partitions.

Computation per sample (C=256 in/out channels, width=64, 56x56 spatial, E=4):
  r1 = groupmean(sigmoid(r1_W @ mean_hw(x) + r1_b))          routing 1
  h1 = relu(bn1(combine(r1, ew1) @ x))                       1x1 CondConv
  r2 = groupmean(sigmoid(r2_W @ mean_hw(h1) + r2_b))         routing 2
  h2 = relu(bn2(conv3x3(combine(r2, ew2), h1)))              3x3 CondConv
  out = relu(bn3(w3 @ h2) + x)                               1x1 + residual

x is cast to bf16 on the host and the output is written bf16 and upcast on
the host, halving both directions of HBM traffic.  BN scales are folded into
the expert weights host-side; BN biases ride the per-partition bias port of
the eviction ops.  The residual add is folded into conv3 as an identity-
weight matmul accumulating into the same PSUM bank.  Routing stays fp32.

Scheduling notes:
- Weight-stationary inner loops (one LDWEIGHTS per stationary covering a
  group of spatial chunks) keep the PE at its ~190ns/448-col issue rate.
- Constants ride 3 packed DMAs; DMA descriptor issues are throttled by a
  global 8-deep completion-semaphore rotation, so descriptor count is kept
  low and issue order matches need order.
- Pooled means are split between DVE column-reduces and in-place ACT
  copies with the fp32 accumulator sidecar, so no single queue
  serializes the routing chain (Pool-engine streaming ops turned out to
  stall concurrent DVE ops and are avoided).
- Pair-1's conv1 is interleaved into pair-0's conv2 window to shorten the
  pipeline fill; pair-0's conv1 runs per-sample halves as its x tiles land.
"""

import sys

for _p in ("/opt/trn_rl_repo",):
    if _p not in sys.path:
        sys.path.insert(0, _p)

import ml_dtypes
import numpy as np

import concourse.bass as bass
import concourse.tile as tile
from concourse import bacc, mybir
from concourse.bass_utils import run_bass_kernel_spmd

dt = mybir.dt
AF = mybir.ActivationFunctionType
ALU = mybir.AluOpType

N_CORES = 8
B, C, HW, S = 64, 256, 56, 56 * 56          # batch, channels, spatial
WD, E, D = 64, 4, 256                        # width, experts, routing interm
BPC = B // N_CORES                           # samples per core (8)
PAIRS = BPC // 2
EPS = 1e-5
NCH = 7                                      # spatial chunks (8 rows x 56 = 448)
CH = S // NCH                                # 448
PW = HW + 2                                  # padded row width 58

# packed-constant column offsets (see _prep_consts)
R1, G, BI, E1 = 0, 512, 520, 528             # in pk32a [128, 1040] f32
R2, EY = 0, 256                              # in pk32b [128, 320]  f32
EW2, W3, I1 = 0, 1152, 1408                  # in pkbf  [128, 1536] bf16
R1B = 0                                      # in pkr   [128, 512]  bf16

_cache = {}


def _build():
    nc = bacc.Bacc("TRN2", target_bir_lowering=False, debug=False,
                   num_devices=N_CORES)
    f32, bf16 = dt.float32, dt.bfloat16

    x_d = nc.dram_tensor("x", [BPC, C, S], bf16, kind="ExternalInput").ap()
    pk32a_d = nc.dram_tensor("pk32a", [128, 1040], f32, kind="ExternalInput").ap()
    pk32b_d = nc.dram_tensor("pk32b", [128, 320], f32, kind="ExternalInput").ap()
    pkr_d = nc.dram_tensor("pkr", [128, 512], bf16, kind="ExternalInput").ap()
    pkbf_d = nc.dram_tensor("pkbf", [128, 1536], bf16, kind="ExternalInput").ap()
    sm4_d = nc.dram_tensor("sm4", [4, 388], f32, kind="ExternalInput").ap()
    out_d = nc.dram_tensor("out", [BPC, C, S], bf16, kind="ExternalOutput").ap()

    with tile.TileContext(nc) as tc:
        with tc.tile_pool(name="const", bufs=1) as cp, \
             tc.tile_pool(name="pers", bufs=1) as pp, \
             tc.tile_pool(name="xp", bufs=12) as xp, \
             tc.tile_pool(name="h2p", bufs=2) as h2p, \
             tc.tile_pool(name="outp", bufs=3) as outp, \
             tc.tile_pool(name="small", bufs=2) as sp, \
             tc.tile_pool(name="poolscr", bufs=3) as plp, \
             tc.tile_pool(name="c12ps", bufs=2, space="PSUM") as c12ps, \
             tc.tile_pool(name="c3ps", bufs=5, space="PSUM") as c3ps, \
             tc.tile_pool(name="rps", bufs=1, space="PSUM") as rps:

            state = {}

            def stA_dma(p, ks=range(4)):
                sa, sb = 2 * p, 2 * p + 1
                locs = ((sa, 0), (sa, 1), (sb, 0), (sb, 1))
                xt = state.setdefault(("xt", p), [None] * 4)
                for k in ks:
                    s, h = locs[k]
                    t = xp.tile([128, S], bf16, tag="xt", name=f"xt_{p}_{k}")
                    nc.sync.dma_start(t[:], x_d[s, 128 * h:128 * h + 128, :])
                    xt[k] = t

            # ---- persistent-buffer memsets first (gpsimd queue) ----
            h1p, w1sb, w2sb = [], [], []
            for q in range(2):
                h1p.append(pp.tile([128, PW * PW], bf16, tag=f"h1p{q}",
                                   name=f"h1p{q}"))
                w1sb.append(pp.tile([128, 512], bf16, tag=f"w1sb{q}",
                                    name=f"w1sb{q}"))
                w2sb.append(pp.tile([128, 1152], bf16, tag=f"w2sb{q}",
                                    name=f"w2sb{q}"))
            for q in range(2):
                tv = w1sb[q][:].rearrange("p (c m) -> p c m", m=128)
                nc.gpsimd.memset(tv[:, 0:2, 64:128], 0.0)
                nc.gpsimd.memset(tv[:, 2:4, 0:64], 0.0)
                tv = h1p[q][:].rearrange("p (r c) -> p r c", r=PW)
                nc.gpsimd.memset(tv[:, 0:1, :], 0.0)
                nc.gpsimd.memset(tv[:, PW - 1:PW, :], 0.0)
                nc.gpsimd.memset(tv[:, :, 0:1], 0.0)
                nc.gpsimd.memset(tv[:, :, PW - 1:PW], 0.0)
                tv = w2sb[q][:].rearrange("p (t m) -> p t m", m=128)
                nc.gpsimd.memset(tv[0:64, :, 64:128], 0.0)
                nc.gpsimd.memset(tv[64:128, :, 0:64], 0.0)

            # ---- DMAs in need-order on the sync queue (transfers
            # complete roughly in issue order at ~2us per x tile) ----
            stA_dma(0, (0, 1))                      # sample-A tiles
            pk32a = cp.tile([128, 1040], f32, tag="pk32a")
            nc.sync.dma_start(pk32a[:], pk32a_d)    # routing+w1 consts
            sm4 = cp.tile([4, 388], f32, tag="sm4")
            nc.sync.dma_start(sm4[:], sm4_d[:])     # routing group masks
            stA_dma(0, (2, 3))                      # sample-B tiles
            pk32b = cp.tile([128, 320], f32, tag="pk32b")
            nc.sync.dma_start(pk32b[:], pk32b_d)    # routing2 consts
            pkr = cp.tile([128, 512], bf16, tag="pkr")
            nc.sync.dma_start(pkr[:], pkr_d)        # bf16 r1 weights
            stA_dma(1, (0, 1))                      # pair-1 sample-A
            pkbf = cp.tile([128, 1536], bf16, tag="pkbf")
            nc.sync.dma_start(pkbf[:], pkbf_d)      # conv2/conv3 weights
            stA_dma(1, (2, 3))                      # pair-1 sample-B
            EYE4 = sm4[:, 0:4]
            ONES4 = sm4[:, 4:132]
            E01 = sm4[:, 132:260]
            E23 = sm4[:, 260:388]

            # ================= stage emitters =================
            # Pooled column-sums, spread across engines by mode.
            def stA_pool(p, k, mode):
                if ("p1", p) not in state:
                    state[("p1", p)] = sp.tile([128, 4], f32, tag="p1",
                                               name=f"p1_{p}")
                    state[("p1b", p)] = sp.tile([128, 4], bf16, tag="p1b",
                                                name=f"p1b_{p}")
                p1 = state[("p1", p)]
                p1b = state[("p1b", p)]
                xt = state[("xt", p)]
                col = (0, 2, 1, 3)[k]
                if mode == "pool":
                    l1 = plp.tile([128, S // 2], bf16, tag="pl1",
                                  name=f"pl1_{p}_{k}")
                    nc.gpsimd.tensor_tensor(
                        l1[:], xt[k][:, 0:S // 2], xt[k][:, S // 2:S],
                        op=ALU.add)
                    nc.vector.tensor_reduce(p1[:, col:col + 1], l1[:],
                                            axis=mybir.AxisListType.X,
                                            op=ALU.add)
                elif mode == "dve":
                    nc.vector.tensor_reduce(p1[:, col:col + 1], xt[k][:],
                                            axis=mybir.AxisListType.X,
                                            op=ALU.add)
                else:
                    nc.scalar.activation(xt[k][:], xt[k][:], AF.Copy,
                                         accum_out=p1[:, col:col + 1])
                nc.vector.tensor_copy(p1b[:, col:col + 1],
                                      p1[:, col:col + 1])

            def stA_route(p):
                p1b = state[("p1b", p)]
                t1sb = []
                for h in range(2):
                    tps = rps.tile([128, 2], f32, tag="rps")
                    for c in range(2):
                        nc.tensor.matmul(
                            tps[:],
                            pkr[:, R1B + 256 * c + 128 * h:
                                R1B + 256 * c + 128 * h + 128],
                            p1b[:, 2 * c:2 * c + 2],
                            start=(c == 0), stop=(c == 1))
                    t = sp.tile([128, 2], f32, tag=f"t1sb{h}")
                    nc.scalar.activation(t[:], tps[:], AF.Sigmoid,
                                         bias=pk32a[:, BI + h:BI + h + 1],
                                         scale=1.0)
                    t1sb.append(t)
                r1ps = rps.tile([4, 2], f32, tag="rps")
                for h in range(2):
                    nc.tensor.matmul(r1ps[:],
                                     pk32a[:, G + 4 * h:G + 4 * h + 4],
                                     t1sb[h][:], start=(h == 0), stop=(h == 1))
                r1sb = sp.tile([4, 2], f32, tag="r1sb")
                nc.vector.tensor_copy(r1sb[:], r1ps[:])
                diag = sp.tile([4, 8], f32, tag="diag")
                for sl in range(2):
                    nc.vector.tensor_scalar(diag[:, 4 * sl:4 * sl + 4], EYE4,
                                            r1sb[:, sl:sl + 1], None,
                                            op0=ALU.mult)
                rbp = rps.tile([128, 8], f32, tag="rps")
                nc.tensor.matmul(rbp[:], ONES4, diag[:], start=True, stop=True)
                rbc = sp.tile([128, 8], f32, tag="rbc")
                nc.vector.tensor_copy(rbc[:], rbp[:])
                state[("rbc", p)] = rbc

            def stA_w1(p, sl):
                rbc = state[("rbc", p)]
                scr = sp.tile([128, 128], f32, tag="w1scr",
                              name=f"w1scr_{p}_{sl}")
                for e in range(E):
                    if e == 0:
                        nc.vector.tensor_scalar(
                            scr[:], pk32a[:, E1:E1 + 128],
                            rbc[:, 4 * sl:4 * sl + 1],
                            None, op0=ALU.mult)
                    else:
                        nc.vector.scalar_tensor_tensor(
                            scr[:], pk32a[:, E1 + 128 * e:E1 + 128 * e + 128],
                            rbc[:, 4 * sl + e:4 * sl + e + 1], scr[:],
                            op0=ALU.mult, op1=ALU.add)
                w1v = w1sb[p % 2][:].rearrange("p (c m) -> p c m", m=128)
                dst = w1v[:, 2 * sl:2 * sl + 2, 64 * sl:64 * sl + 64]
                nc.scalar.copy(dst, scr[:].rearrange("p (c o) -> p c o", o=64))

            # Stage B(p): conv1 + bn1 + pooled2 + routing2 + w2.
            # Weight-stationary: one LDWEIGHTS per contraction chunk for
            # all of js; evictions alternate ACT/DVE (both carry the
            # pooled-h1 accumulator).
            def _c1_evict(p, j, ps):
                q = p % 2
                h1v = h1p[q][:].rearrange("p (r c) -> p r c", r=PW)
                acc1 = state[("acc1", p)]
                dstv = h1v[:, 1 + 8 * j:9 + 8 * j, 1:57]
                nc.scalar.activation(
                    dstv, ps[:], AF.Relu, bias=pk32a[:, BI + 4:BI + 5],
                    scale=1.0, accum_out=acc1[:, j:j + 1])

            def stB_conv1(p, js):
                q = p % 2
                xt = state[("xt", p)]
                if ("acc1", p) not in state:
                    state[("acc1", p)] = sp.tile([128, NCH], f32,
                                                 tag="acc1", name=f"acc1_{p}")
                pss = {j: c12ps.tile([128, CH], f32, tag="c12",
                                     name=f"c1_{p}_{j}") for j in js}
                for c in range(4):
                    for j in js:
                        nc.tensor.matmul(
                            pss[j][:], w1sb[q][:, 128 * c:128 * c + 128],
                            xt[c][:, CH * j:CH * j + CH],
                            start=(c == 0), stop=(c == 3))
                for j in js:
                    _c1_evict(p, j, pss[j])

            def stB_pool2(p):
                acc1 = state[("acc1", p)]
                p2 = sp.tile([128, 1], f32, tag="p2")
                nc.vector.tensor_reduce(p2[:], acc1[:],
                                        axis=mybir.AxisListType.X, op=ALU.add)
                t2sb = []
                for h in range(2):
                    tps = rps.tile([128, 2], f32, tag="rps")
                    for sl in range(2):
                        po = 64 * sl
                        nc.tensor.matmul(
                            tps[:, sl:sl + 1],
                            pk32b[po:po + 64, R2 + 128 * h:R2 + 128 * h + 128],
                            p2[po:po + 64, :], start=True, stop=True)
                    t = sp.tile([128, 2], f32, tag=f"t2sb{h}")
                    nc.scalar.activation(t[:], tps[:], AF.Sigmoid,
                                         bias=pk32a[:, BI + 2 + h:BI + 3 + h],
                                         scale=1.0)
                    t2sb.append(t)
                state[("t2sb", p)] = t2sb

            def stB_r2(p):
                t2sb = state[("t2sb", p)]
                r2ps = rps.tile([4, 2], f32, tag="rps")
                for h in range(2):
                    nc.tensor.matmul(r2ps[:],
                                     pk32a[:, G + 4 * h:G + 4 * h + 4],
                                     t2sb[h][:], start=(h == 0), stop=(h == 1))
                r2sb = sp.tile([4, 2], f32, tag="r2sb")
                nc.vector.tensor_copy(r2sb[:], r2ps[:])
                cols = []
                for c, sel in enumerate((E01, E23)):
                    cps = rps.tile([128, 2], f32, tag="rps")
                    nc.tensor.matmul(cps[:], sel, r2sb[:], start=True, stop=True)
                    t = sp.tile([128, 2], f32, tag=f"cols{c}")
                    nc.vector.tensor_copy(t[:], cps[:])
                    cols.append(t)
                state[("cols", p)] = cols

            def stB_rl(p):
                cols = state[("cols", p)]
                rl = sp.tile([128, 256], bf16, tag="rl")
                for c in range(2):
                    nc.vector.tensor_scalar(
                        rl[:, 128 * c:128 * c + 64], pk32b[:, EY:EY + 64],
                        cols[c][:, 0:1], None, op0=ALU.mult)
                    nc.vector.tensor_scalar(
                        rl[:, 128 * c + 64:128 * c + 128],
                        pk32b[:, EY:EY + 64],
                        cols[c][:, 1:2], None, op0=ALU.mult)
                state[("rl", p)] = rl

            def stB_w2(p):
                q = p % 2
                rl = state[("rl", p)]
                w2v = w2sb[q][:].rearrange("p (t m) -> p t m", m=128)
                for g0, g1 in ((0, 512), (512, 576)):
                    wps = rps.tile([128, g1 - g0], f32, tag="rps")
                    for c in range(2):
                        nc.tensor.matmul(
                            wps[:], rl[:, 128 * c:128 * c + 128],
                            pkbf[:, EW2 + 576 * c + g0:EW2 + 576 * c + g1],
                            start=(c == 0), stop=(c == 1))
                    wpv = wps[:].rearrange("p (t o) -> p t o", o=64)
                    t0, t1 = g0 // 64, g1 // 64
                    nc.vector.tensor_copy(w2v[0:64, t0:t1, 0:64], wpv[0:64])
                    nc.vector.tensor_copy(w2v[64:128, t0:t1, 64:128], wpv[64:128])

            # Stage C(p) part 1: conv2 + bn2 -> h2 (tap-stationary).
            def stC_conv2(p, js):
                q = p % 2
                h1v = h1p[q][:].rearrange("p (r c) -> p r c", r=PW)
                if ("h2", p) not in state:
                    state[("h2", p)] = h2p.tile([128, S], bf16,
                                                tag="h2", name=f"h2_{p}")
                h2 = state[("h2", p)]
                pss = {j: c12ps.tile([128, CH], f32, tag="c12",
                                     name=f"c2_{p}_{j}") for j in js}
                for t9 in range(9):
                    kh, kw = divmod(t9, 3)
                    for j in js:
                        nc.tensor.matmul(
                            pss[j][:], w2sb[q][:, 128 * t9:128 * t9 + 128],
                            h1v[:, 8 * j + kh:8 * j + kh + 8, kw:kw + 56],
                            start=(t9 == 0), stop=(t9 == 8))
                for j in js:
                    nc.scalar.activation(h2[:, CH * j:CH * j + CH], pss[j][:],
                                         AF.Relu,
                                         bias=pk32a[:, BI + 5:BI + 6],
                                         scale=1.0)

            # Stage C(p) part 2: conv3 chunks of one (sample, out-half)
            # group; w3-stationary then identity-stationary runs.
            def stC_conv3(p, sl, h, js=range(NCH)):
                h2 = state[("h2", p)]
                xt = state[("xt", p)]
                po = 64 * sl
                key = ("ost", p, sl, h)
                if key not in state:
                    state[key] = outp.tile([128, S], bf16, tag="ost",
                                           name=f"ost_{p}_{sl}_{h}")
                ost = state[key]
                pss = {j: c3ps.tile([128, CH], f32, tag="c3",
                                    name=f"c3_{p}_{sl}_{h}_{j}") for j in js}
                for j in js:
                    nc.tensor.matmul(
                        pss[j][:],
                        pkbf[po:po + 64, W3 + 128 * h:W3 + 128 * h + 128],
                        h2[po:po + 64, CH * j:CH * j + CH],
                        start=True, stop=False)
                for j in js:
                    nc.tensor.matmul(
                        pss[j][:], pkbf[:, I1:I1 + 128],
                        xt[2 * sl + h][:, CH * j:CH * j + CH],
                        start=False, stop=True)
                for j in js:
                    if h == 1:
                        nc.scalar.activation(
                            ost[:, CH * j:CH * j + CH], pss[j][:], AF.Relu,
                            bias=pk32a[:, BI + 6 + h:BI + 7 + h], scale=1.0)
                    else:
                        nc.vector.tensor_scalar(
                            ost[:, CH * j:CH * j + CH], pss[j][:],
                            pk32a[:, BI + 6 + h:BI + 7 + h], 0.0,
                            op0=ALU.add, op1=ALU.max)
                if p == PAIRS - 1:
                    lo = CH * (js[0] - js[0] % 2)
                    hi = CH * (js[-1] + 1)
                    nc.sync.dma_start(
                        out_d[2 * p + sl, 128 * h:128 * h + 128, lo:hi],
                        ost[:, lo:hi])
                elif js[-1] == NCH - 1:
                    nc.sync.dma_start(
                        out_d[2 * p + sl, 128 * h:128 * h + 128, :], ost[:])

            # ================= pipelined emission =================
            def c3g(p, sl, h, part):
                js = ((0, 1), (2, 3), (4, 5), (6,))[part]
                stC_conv3(p, sl, h, js)

            # pair 0 prologue: per-sample routing so sample A's weights
            # are ready while sample B's x tiles are still arriving, then
            # conv1 with the A-channel half of every chunk first.
            p1_0 = sp.tile([128, 4], f32, tag="p1", name="p1_0")
            state[("p1", 0)] = p1_0
            p1b_0 = sp.tile([128, 4], bf16, tag="p1b", name="p1b_0")
            state[("p1b", 0)] = p1b_0
            for sl in range(2):
                stA_pool(0, 2 * sl, "dve")
                stA_pool(0, 2 * sl + 1, "act")
                t1sb = []
                for h in range(2):
                    tps = rps.tile([128, 1], f32, tag="rps")
                    for c in range(2):
                        nc.tensor.matmul(
                            tps[:],
                            pkr[:, R1B + 256 * c + 128 * h:
                                R1B + 256 * c + 128 * h + 128],
                            p1b_0[:, sl + 2 * c:sl + 2 * c + 1],
                            start=(c == 0), stop=(c == 1))
                    t = sp.tile([128, 1], f32, tag=f"t1sb{h}",
                                name=f"t1s0_{sl}_{h}")
                    nc.scalar.activation(t[:], tps[:], AF.Sigmoid,
                                         bias=pk32a[:, BI + h:BI + h + 1],
                                         scale=1.0)
                    t1sb.append(t)
                r1ps = rps.tile([4, 1], f32, tag="rps")
                for h in range(2):
                    nc.tensor.matmul(r1ps[:],
                                     pk32a[:, G + 4 * h:G + 4 * h + 4],
                                     t1sb[h][:], start=(h == 0), stop=(h == 1))
                r1sb = sp.tile([4, 1], f32, tag="r1sb",
                               name=f"r1s0_{sl}")
                nc.vector.tensor_copy(r1sb[:], r1ps[:])
                diag = sp.tile([4, 4], f32, tag="diag",
                               name=f"diag0_{sl}")
                nc.vector.tensor_scalar(diag[:], EYE4, r1sb[:], None,
                                        op0=ALU.mult)
                rbp = rps.tile([128, 4], f32, tag="rps")
                nc.tensor.matmul(rbp[:], ONES4, diag[:], start=True, stop=True)
                if ("rbc", 0) not in state:
                    state[("rbc", 0)] = sp.tile([128, 8], f32,
                                                tag="rbc", name="rbc_0")
                nc.vector.tensor_copy(
                    state[("rbc", 0)][:, 4 * sl:4 * sl + 4], rbp[:])
                stA_w1(0, sl)
                if sl == 0:
                    # A-channel half of conv1 over all 7 chunks keeps the
                    # PE busy while sample B's tiles/routing catch up.
                    xt0 = state[("xt", 0)]
                    state[("acc1", 0)] = sp.tile([128, NCH], f32,
                                                 tag="acc1", name="acc1_0")
                    c1ps = []
                    for j in range(NCH):
                        pool = c3ps if j < 5 else c12ps
                        tag = "c3" if j < 5 else "c12"
                        c1ps.append(pool.tile([128, CH], f32, tag=tag,
                                              name=f"c1p0_{j}"))
                    for c in range(2):
                        for j in range(NCH):
                            nc.tensor.matmul(
                                c1ps[j][:], w1sb[0][:, 128 * c:128 * c + 128],
                                xt0[c][:, CH * j:CH * j + CH],
                                start=(c == 0), stop=False)
            # B-channel half + evictions
            xt0 = state[("xt", 0)]
            for c in range(2, 4):
                for j in range(NCH):
                    nc.tensor.matmul(
                        c1ps[j][:], w1sb[0][:, 128 * c:128 * c + 128],
                        xt0[c][:, CH * j:CH * j + CH],
                        start=False, stop=(c == 3))
            for j in range(NCH):
                _c1_evict(0, j, c1ps[j])
            # pair-1 pooling rides the idle Pool engine so DVE/ACT stay
            # clear for pair-0's routing chain and evictions
            stA_pool(1, 0, "dve")
            stA_pool(1, 1, "act")

            for p in range(PAIRS):
                prv = p - 1 >= 0
                nxt = p + 1 < PAIRS

                def c3(sl, h, part):
                    if prv:
                        c3g(p - 1, sl, h, part)

                # -- window 1 -- (conv1 of pairs 0/1 runs earlier)
                if p > 0:
                    c3(0, 0, 0)
                    if nxt:
                        stA_dma(p + 1, (0,))
                    c3(0, 0, 1)
                    if nxt:
                        stA_dma(p + 1, (1,))
                    c3(0, 0, 2)
                    if nxt:
                        stA_dma(p + 1, (2,))
                    c3(0, 0, 3)
                    if nxt:
                        stA_dma(p + 1, (3,))
                    c3(0, 1, 0)
                    c3(0, 1, 1)
                    c3(0, 1, 2)
                    c3(0, 1, 3)
                    if p >= 2:
                        stB_conv1(p, (0, 1))
                        if nxt:
                            stA_pool(p + 1, 0, "dve")
                        stB_conv1(p, (2, 3))
                        if nxt:
                            stA_pool(p + 1, 1, "act")
                        stB_conv1(p, (4, 5))
                        stB_conv1(p, (6,))
                    elif nxt:
                        stA_pool(p + 1, 0, "dve")
                        stA_pool(p + 1, 1, "act")
                # -- window 2 --
                stB_pool2(p)
                c3(1, 0, 0)
                stB_r2(p)
                c3(1, 0, 1)
                stB_rl(p)
                c3(1, 0, 2)
                stB_w2(p)
                c3(1, 0, 3)
                if p == 0:
                    stA_pool(1, 2, "dve")
                    stA_pool(1, 3, "act")
                    stA_route(1)
                    stA_w1(1, 0)
                    stA_w1(1, 1)
                elif nxt:
                    stA_pool(p + 1, 2, "act")
                # -- window 3 --
                stC_conv2(p, (0, 1))
                c3(1, 1, 0)
                if p == 0:
                    stB_conv1(1, (0, 1))
                elif nxt:
                    stA_pool(p + 1, 3, "dve")
                stC_conv2(p, (2, 3))
                c3(1, 1, 1)
                if p == 0:
                    stB_conv1(1, (2, 3))
                elif nxt:
                    stA_route(p + 1)
                stC_conv2(p, (4, 5))
                c3(1, 1, 2)
                if p == 0:
                    stB_conv1(1, (4, 5))
                elif nxt:
                    stA_w1(p + 1, 0)
                stC_conv2(p, (6,))
                c3(1, 1, 3)
                if p == 0:
                    stB_conv1(1, (6,))
                elif nxt:
                    stA_w1(p + 1, 1)
            # epilogue: last pair's conv3, two groups interleaved at a time
            for grps in (((0, 0), (0, 1)), ((1, 0), (1, 1))):
                for part in range(4):
                    for sl, h in grps:
                        c3g(PAIRS - 1, sl, h, part)

    nc.compile()
    return nc


def _prep_consts(r1_W, r1_b, ew1, bn1_g, bn1_b, bn1_m, bn1_v,
                 r2_W, r2_b, ew2, bn2_g, bn2_b, bn2_m, bn2_v,
                 w3, bn3_g, bn3_b, bn3_m, bn3_v):
    f = np.float32
    s1 = (bn1_g / np.sqrt(bn1_v + EPS)).astype(f)
    b1 = (bn1_b - bn1_m * s1).astype(f)
    s2 = (bn2_g / np.sqrt(bn2_v + EPS)).astype(f)
    b2 = (bn2_b - bn2_m * s2).astype(f)
    s3 = (bn3_g / np.sqrt(bn3_v + EPS)).astype(f)
    b3 = (bn3_b - bn3_m * s3).astype(f)

    # ew1c [e, i128, (chunk, o)]  (bn1 scale folded)
    ew1s = ew1.reshape(E, WD, C) * s1[None, :, None]          # [e, o, i]
    ew1c = np.ascontiguousarray(
        ew1s.transpose(0, 2, 1)                                # [e, i, o]
        .reshape(E, 2, 128, WD)                                # [e, c, i128, o]
        .transpose(0, 2, 1, 3)                                 # [e, i128, c, o]
        .reshape(E, 128, 128)).astype(f)

    # ew2c [chunk, (e2, i), (tap, o)]  (bn2 scale folded)
    ew2s = ew2.reshape(E, WD, WD, 9) * s2[None, :, None, None]  # [e, o, i, t]
    ew2c = np.ascontiguousarray(
        ew2s.transpose(0, 2, 3, 1)                             # [e, i, t, o]
        .reshape(2, 128, 9 * WD))

    w3h = (w3 * s3[:, None]).T.astype(np.float32)              # [i 64, o 256]
    w3t = np.concatenate([w3h, w3h], 0)

    i128 = np.eye(128, dtype=f)
    r1wt = np.ascontiguousarray((r1_W.T / S).reshape(2, 128, D)).astype(f)
    r2h = (r2_W.T / S).astype(f)                               # [64, 256]
    r2wt = np.concatenate([r2h, r2h], 0)

    g = np.zeros((D, E), f)
    g[np.arange(D), np.arange(D) // WD] = 1.0 / WD
    gsel = np.ascontiguousarray(g.reshape(2, 128, E))

    sm4 = np.zeros((4, 388), f)
    sm4[:, 0:4] = np.eye(4, dtype=f)
    sm4[:, 4:132] = 1.0
    sm4[0, 132:196] = 1.0
    sm4[1, 196:260] = 1.0
    sm4[2, 260:324] = 1.0
    sm4[3, 324:388] = 1.0

    eye2 = np.concatenate([np.eye(WD, dtype=f), np.eye(WD, dtype=f)], 0)

    bias = np.zeros((128, 8), f)
    bias[:, 0] = r1_b[0:128]
    bias[:, 1] = r1_b[128:256]
    bias[:, 2] = r2_b[0:128]
    bias[:, 3] = r2_b[128:256]
    bias[:, 4] = np.concatenate([b1, b1])
    bias[:, 5] = np.concatenate([b2, b2])
    bias[:, 6] = b3[0:128]
    bias[:, 7] = b3[128:256]

    bf = ml_dtypes.bfloat16
    pk32a = np.ascontiguousarray(np.concatenate(
        [r1wt[0], r1wt[1], gsel[0], gsel[1], bias,
         ew1c[0], ew1c[1], ew1c[2], ew1c[3]], axis=1)).astype(f)
    pk32b = np.ascontiguousarray(np.concatenate([r2wt, eye2], axis=1)).astype(f)
    pkr = np.ascontiguousarray(np.concatenate(
        [r1wt[0], r1wt[1]], axis=1)).astype(ml_dtypes.bfloat16)
    pkbf = np.ascontiguousarray(np.concatenate(
        [ew2c[0], ew2c[1], w3t, i128], axis=1)).astype(bf)

    return dict(pk32a=pk32a, pk32b=pk32b, pkr=pkr, pkbf=pkbf, sm4=sm4)


def kernel(x, **weights):
    if "nc" not in _cache:
        _cache["nc"] = _build()
    nc = _cache["nc"]
    consts = _prep_consts(**{k: np.asarray(v) for k, v in weights.items()})
    xf = np.asarray(x, dtype=np.float32).reshape(B, C, S).astype(
        ml_dtypes.bfloat16)
    in_maps = []
    for c in range(N_CORES):
        m = {"x": np.ascontiguousarray(xf[BPC * c:BPC * (c + 1)])}
        m.update(consts)
        in_maps.append(m)
    res = run_bass_kernel_spmd(nc, in_maps, core_ids=list(range(N_CORES)),
                               **_cache.get("run_kwargs", {}))
    _cache["last_res"] = res
    out = np.concatenate(
        [res.results[c]["out"][None] for c in range(N_CORES)], 0)
    return out.astype(np.float32).reshape(B, C, HW, HW)



# revision 4
# speedup vs baseline: 1.0152x; 1.0152x over previous
"""Trainium2 Bass kernel for nn_MoEBottleneck (moe_routing).

Data-parallel over batch: 64 samples sharded 8-per-core across 8 NeuronCores.
Per core, samples are processed in pairs packed onto the 128 SBUF partitions.

Computation per sample (C=256 in/out channels, width=64, 56x56 spatial, E=4):
  r1 = groupmean(sigmoid(r1_W @ mean_hw(x) + r1_b))          routing 1
  h1 = relu(bn1(combine(r1, ew1) @ x))                       1x1 CondConv
  r2 = groupmean(sigmoid(r2_W @ mean_hw(h1) + r2_b))         routing 2
  h2 = relu(bn2(conv3x3(combine(r2, ew2), h1)))              3x3 CondConv
  out = relu(bn3(w3 @ h2) + x)                               1x1 + residual

x is cast to bf16 on the host and the output is written bf16 and upcast on
the host, halving both directions of HBM traffic.  BN scales are folded into
the expert weights host-side; BN biases ride the per-partition bias port of
the eviction ops.  The residual add is folded into conv3 as an identity-
weight matmul accumulating into the same PSUM bank.  Routing stays fp32.

Scheduling notes:
- Weight-stationary inner loops (one LDWEIGHTS per stationary covering a
  group of spatial chunks) keep the PE at its ~190ns/448-col issue rate.
- Constants ride 3 packed DMAs; DMA descriptor issues are throttled by a
  global 8-deep completion-semaphore rotation, so descriptor count is kept
  low and issue order matches need order.
- Pooled means are split between DVE column-reduces and in-place ACT
  copies with the fp32 accumulator sidecar, so no single queue
  serializes the routing chain (Pool-engine streaming ops turned out to
  stall concurrent DVE ops and are avoided).
- Pair-1's conv1 is interleaved into pair-0's conv2 window to shorten the
  pipeline fill; pair-0's conv1 runs per-sample halves as its x tiles land.
"""

import sys

for _p in ("/opt/trn_rl_repo",):
    if _p not in sys.path:
        sys.path.insert(0, _p)

import ml_dtypes
import numpy as np

import concourse.bass as bass
import concourse.tile as tile
from concourse import bacc, mybir
from concourse.bass_utils import run_bass_kernel_spmd

dt = mybir.dt
AF = mybir.ActivationFunctionType
ALU = mybir.AluOpType

N_CORES = 8
B, C, HW, S = 64, 256, 56, 56 * 56          # batch, channels, spatial
WD, E, D = 64, 4, 256                        # width, experts, routing interm
BPC = B // N_CORES                           # samples per core (8)
PAIRS = BPC // 2
EPS = 1e-5
NCH = 7                                      # spatial chunks (8 rows x 56 = 448)
CH = S // NCH                                # 448
PW = HW + 2                                  # padded row width 58

# packed-constant column offsets (see _prep_consts)
R1, G, BI, E1 = 0, 512, 520, 528             # in pk32a [128, 1040] f32
R2, EY = 0, 256                              # in pk32b [128, 320]  f32
EW2, W3, I1 = 0, 1152, 1408                  # in pkbf  [128, 1536] bf16
R1B = 0                                      # in pkr   [128, 512]  bf16

_cache = {}


def _build():
    nc = bacc.Bacc("TRN2", target_bir_lowering=False, debug=False,
                   num_devices=N_CORES)
    f32, bf16 = dt.float32, dt.bfloat16

    x_d = nc.dram_tensor("x", [BPC, C, S], bf16, kind="ExternalInput").ap()
    pk32a_d = nc.dram_tensor("pk32a", [128, 1040], f32, kind="ExternalInput").ap()
    pk32b_d = nc.dram_tensor("pk32b", [128, 320], f32, kind="ExternalInput").ap()
    pkr_d = nc.dram_tensor("pkr", [128, 512], bf16, kind="ExternalInput").ap()
    pkbf_d = nc.dram_tensor("pkbf", [128, 1536], bf16, kind="ExternalInput").ap()
    sm4_d = nc.dram_tensor("sm4", [4, 388], f32, kind="ExternalInput").ap()
    out_d = nc.dram_tensor("out", [BPC, C, S], bf16, kind="ExternalOutput").ap()

    with tile.TileContext(nc) as tc:
        with tc.tile_pool(name="const", bufs=1) as cp, \
             tc.tile_pool(name="pers", bufs=1) as pp, \
             tc.tile_pool(name="xp", bufs=16) as xp, \
             tc.tile_pool(name="h2p", bufs=2) as h2p, \
             tc.tile_pool(name="outp", bufs=4) as outp, \
             tc.tile_pool(name="small", bufs=2) as sp, \
             tc.tile_pool(name="poolscr", bufs=3) as plp, \
             tc.tile_pool(name="c12ps", bufs=2, space="PSUM") as c12ps, \
             tc.tile_pool(name="c3ps", bufs=5, space="PSUM") as c3ps, \
             tc.tile_pool(name="rps", bufs=1, space="PSUM") as rps:

            state = {}

            def stA_dma(p, ks=range(4)):
                sa, sb = 2 * p, 2 * p + 1
                locs = ((sa, 0), (sa, 1), (sb, 0), (sb, 1))
                xt = state.setdefault(("xt", p), [None] * 4)
                for k in ks:
                    s, h = locs[k]
                    t = xp.tile([128, S], bf16, tag="xt", name=f"xt_{p}_{k}")
                    nc.sync.dma_start(t[:], x_d[s, 128 * h:128 * h + 128, :])
                    xt[k] = t

            # ---- persistent-buffer memsets first (gpsimd queue) ----
            h1p, w1sb, w2sb = [], [], []
            for q in range(2):
                h1p.append(pp.tile([128, PW * PW], bf16, tag=f"h1p{q}",
                                   name=f"h1p{q}"))
                w1sb.append(pp.tile([128, 512], bf16, tag=f"w1sb{q}",
                                    name=f"w1sb{q}"))
                w2sb.append(pp.tile([128, 1152], bf16, tag=f"w2sb{q}",
                                    name=f"w2sb{q}"))
            for q in range(2):
                tv = w1sb[q][:].rearrange("p (c m) -> p c m", m=128)
                nc.gpsimd.memset(tv[:, 0:2, 64:128], 0.0)
                nc.gpsimd.memset(tv[:, 2:4, 0:64], 0.0)
                tv = h1p[q][:].rearrange("p (r c) -> p r c", r=PW)
                nc.gpsimd.memset(tv[:, 0:1, :], 0.0)
                nc.gpsimd.memset(tv[:, PW - 1:PW, :], 0.0)
                nc.gpsimd.memset(tv[:, :, 0:1], 0.0)
                nc.gpsimd.memset(tv[:, :, PW - 1:PW], 0.0)
                tv = w2sb[q][:].rearrange("p (t m) -> p t m", m=128)
                nc.gpsimd.memset(tv[0:64, :, 64:128], 0.0)
                nc.gpsimd.memset(tv[64:128, :, 0:64], 0.0)

            # ---- DMAs in need-order on the sync queue; all 16 x tiles
            # are prefetched upfront (SBUF holds them all) so no mid-
            # stream DMA waits gate later pairs ----
            stA_dma(0, (0, 1))                      # sample-A tiles
            pk32a = cp.tile([128, 1040], f32, tag="pk32a")
            nc.sync.dma_start(pk32a[:], pk32a_d)    # routing+w1 consts
            pkr = cp.tile([128, 512], bf16, tag="pkr")
            nc.sync.dma_start(pkr[:], pkr_d)        # bf16 r1 weights
            sm4 = cp.tile([4, 388], f32, tag="sm4")
            nc.sync.dma_start(sm4[:], sm4_d[:])     # routing group masks
            stA_dma(0, (2, 3))                      # sample-B tiles
            pk32b = cp.tile([128, 320], f32, tag="pk32b")
            nc.sync.dma_start(pk32b[:], pk32b_d)    # routing2 consts
            stA_dma(1, (0, 1))                      # pair-1 sample-A
            pkbf = cp.tile([128, 1536], bf16, tag="pkbf")
            nc.sync.dma_start(pkbf[:], pkbf_d)      # conv2/conv3 weights
            stA_dma(1, (2, 3))                      # pair-1 sample-B
            stA_dma(2, (0, 1))
            stA_dma(2, (2, 3))
            stA_dma(3, (0, 1))
            stA_dma(3, (2, 3))
            EYE4 = sm4[:, 0:4]
            ONES4 = sm4[:, 4:132]
            E01 = sm4[:, 132:260]
            E23 = sm4[:, 260:388]

            # ================= stage emitters =================
            # Pooled column-sums, spread across engines by mode.
            def stA_pool(p, k, mode):
                if ("p1", p) not in state:
                    state[("p1", p)] = sp.tile([128, 4], f32, tag="p1",
                                               name=f"p1_{p}")
                    state[("p1b", p)] = sp.tile([128, 4], bf16, tag="p1b",
                                                name=f"p1b_{p}")
                p1 = state[("p1", p)]
                p1b = state[("p1b", p)]
                xt = state[("xt", p)]
                col = (0, 2, 1, 3)[k]
                if mode == "pool":
                    l1 = plp.tile([128, S // 2], bf16, tag="pl1",
                                  name=f"pl1_{p}_{k}")
                    nc.gpsimd.tensor_tensor(
                        l1[:], xt[k][:, 0:S // 2], xt[k][:, S // 2:S],
                        op=ALU.add)
                    nc.vector.tensor_reduce(p1[:, col:col + 1], l1[:],
                                            axis=mybir.AxisListType.X,
                                            op=ALU.add)
                elif mode == "dve":
                    nc.vector.tensor_reduce(p1[:, col:col + 1], xt[k][:],
                                            axis=mybir.AxisListType.X,
                                            op=ALU.add)
                else:
                    nc.scalar.activation(xt[k][:], xt[k][:], AF.Copy,
                                         accum_out=p1[:, col:col + 1])
                nc.vector.tensor_copy(p1b[:, col:col + 1],
                                      p1[:, col:col + 1])

            def stA_route(p):
                p1b = state[("p1b", p)]
                t1sb = []
                for h in range(2):
                    tps = rps.tile([128, 2], f32, tag="rps")
                    for c in range(2):
                        nc.tensor.matmul(
                            tps[:],
                            pkr[:, R1B + 256 * c + 128 * h:
                                R1B + 256 * c + 128 * h + 128],
                            p1b[:, 2 * c:2 * c + 2],
                            start=(c == 0), stop=(c == 1))
                    t = sp.tile([128, 2], f32, tag=f"t1sb{h}")
                    nc.scalar.activation(t[:], tps[:], AF.Sigmoid,
                                         bias=pk32a[:, BI + h:BI + h + 1],
                                         scale=1.0)
                    t1sb.append(t)
                r1ps = rps.tile([4, 2], f32, tag="rps")
                for h in range(2):
                    nc.tensor.matmul(r1ps[:],
                                     pk32a[:, G + 4 * h:G + 4 * h + 4],
                                     t1sb[h][:], start=(h == 0), stop=(h == 1))
                r1sb = sp.tile([4, 2], f32, tag="r1sb")
                nc.vector.tensor_copy(r1sb[:], r1ps[:])
                diag = sp.tile([4, 8], f32, tag="diag")
                for sl in range(2):
                    nc.vector.tensor_scalar(diag[:, 4 * sl:4 * sl + 4], EYE4,
                                            r1sb[:, sl:sl + 1], None,
                                            op0=ALU.mult)
                rbp = rps.tile([128, 8], f32, tag="rps")
                nc.tensor.matmul(rbp[:], ONES4, diag[:], start=True, stop=True)
                rbc = sp.tile([128, 8], f32, tag="rbc")
                nc.vector.tensor_copy(rbc[:], rbp[:])
                state[("rbc", p)] = rbc

            def stA_w1(p, sl):
                rbc = state[("rbc", p)]
                scr = sp.tile([128, 128], f32, tag="w1scr",
                              name=f"w1scr_{p}_{sl}")
                for e in range(E):
                    if e == 0:
                        nc.vector.tensor_scalar(
                            scr[:], pk32a[:, E1:E1 + 128],
                            rbc[:, 4 * sl:4 * sl + 1],
                            None, op0=ALU.mult)
                    else:
                        nc.vector.scalar_tensor_tensor(
                            scr[:], pk32a[:, E1 + 128 * e:E1 + 128 * e + 128],
                            rbc[:, 4 * sl + e:4 * sl + e + 1], scr[:],
                            op0=ALU.mult, op1=ALU.add)
                w1v = w1sb[p % 2][:].rearrange("p (c m) -> p c m", m=128)
                dst = w1v[:, 2 * sl:2 * sl + 2, 64 * sl:64 * sl + 64]
                nc.scalar.copy(dst, scr[:].rearrange("p (c o) -> p c o", o=64))

            # Stage B(p): conv1 + bn1 + pooled2 + routing2 + w2.
            # Weight-stationary: one LDWEIGHTS per contraction chunk for
            # all of js; evictions alternate ACT/DVE (both carry the
            # pooled-h1 accumulator).
            def _c1_evict(p, j, ps):
                q = p % 2
                h1v = h1p[q][:].rearrange("p (r c) -> p r c", r=PW)
                acc1 = state[("acc1", p)]
                dstv = h1v[:, 1 + 8 * j:9 + 8 * j, 1:57]
                nc.scalar.activation(
                    dstv, ps[:], AF.Relu, bias=pk32a[:, BI + 4:BI + 5],
                    scale=1.0, accum_out=acc1[:, j:j + 1])

            def stB_conv1(p, js):
                q = p % 2
                xt = state[("xt", p)]
                if ("acc1", p) not in state:
                    state[("acc1", p)] = sp.tile([128, NCH], f32,
                                                 tag="acc1", name=f"acc1_{p}")
                pss = {j: c12ps.tile([128, CH], f32, tag="c12",
                                     name=f"c1_{p}_{j}") for j in js}
                for c in range(4):
                    for j in js:
                        nc.tensor.matmul(
                            pss[j][:], w1sb[q][:, 128 * c:128 * c + 128],
                            xt[c][:, CH * j:CH * j + CH],
                            start=(c == 0), stop=(c == 3))
                for j in js:
                    _c1_evict(p, j, pss[j])

            def stB_pool2(p):
                acc1 = state[("acc1", p)]
                p2 = sp.tile([128, 1], f32, tag="p2")
                nc.vector.tensor_reduce(p2[:], acc1[:],
                                        axis=mybir.AxisListType.X, op=ALU.add)
                t2sb = []
                for h in range(2):
                    tps = rps.tile([128, 2], f32, tag="rps")
                    for sl in range(2):
                        po = 64 * sl
                        nc.tensor.matmul(
                            tps[:, sl:sl + 1],
                            pk32b[po:po + 64, R2 + 128 * h:R2 + 128 * h + 128],
                            p2[po:po + 64, :], start=True, stop=True)
                    t = sp.tile([128, 2], f32, tag=f"t2sb{h}")
                    nc.scalar.activation(t[:], tps[:], AF.Sigmoid,
                                         bias=pk32a[:, BI + 2 + h:BI + 3 + h],
                                         scale=1.0)
                    t2sb.append(t)
                state[("t2sb", p)] = t2sb

            def stB_r2(p):
                t2sb = state[("t2sb", p)]
                r2ps = rps.tile([4, 2], f32, tag="rps")
                for h in range(2):
                    nc.tensor.matmul(r2ps[:],
                                     pk32a[:, G + 4 * h:G + 4 * h + 4],
                                     t2sb[h][:], start=(h == 0), stop=(h == 1))
                r2sb = sp.tile([4, 2], f32, tag="r2sb")
                nc.vector.tensor_copy(r2sb[:], r2ps[:])
                cols = []
                for c, sel in enumerate((E01, E23)):
                    cps = rps.tile([128, 2], f32, tag="rps")
                    nc.tensor.matmul(cps[:], sel, r2sb[:], start=True, stop=True)
                    t = sp.tile([128, 2], f32, tag=f"cols{c}")
                    nc.vector.tensor_copy(t[:], cps[:])
                    cols.append(t)
                state[("cols", p)] = cols

            def stB_rl(p):
                cols = state[("cols", p)]
                rl = sp.tile([128, 256], bf16, tag="rl")
                for c in range(2):
                    nc.vector.tensor_scalar(
                        rl[:, 128 * c:128 * c + 64], pk32b[:, EY:EY + 64],
                        cols[c][:, 0:1], None, op0=ALU.mult)
                    nc.vector.tensor_scalar(
                        rl[:, 128 * c + 64:128 * c + 128],
                        pk32b[:, EY:EY + 64],
                        cols[c][:, 1:2], None, op0=ALU.mult)
                state[("rl", p)] = rl

            def stB_w2(p):
                q = p % 2
                rl = state[("rl", p)]
                w2v = w2sb[q][:].rearrange("p (t m) -> p t m", m=128)
                for g0, g1 in ((0, 512), (512, 576)):
                    wps = rps.tile([128, g1 - g0], f32, tag="rps")
                    for c in range(2):
                        nc.tensor.matmul(
                            wps[:], rl[:, 128 * c:128 * c + 128],
                            pkbf[:, EW2 + 576 * c + g0:EW2 + 576 * c + g1],
                            start=(c == 0), stop=(c == 1))
                    wpv = wps[:].rearrange("p (t o) -> p t o", o=64)
                    t0, t1 = g0 // 64, g1 // 64
                    nc.vector.tensor_copy(w2v[0:64, t0:t1, 0:64], wpv[0:64])
                    nc.vector.tensor_copy(w2v[64:128, t0:t1, 64:128], wpv[64:128])

            # Stage C(p) part 1: conv2 + bn2 -> h2 (tap-stationary).
            def stC_conv2(p, js):
                q = p % 2
                h1v = h1p[q][:].rearrange("p (r c) -> p r c", r=PW)
                if ("h2", p) not in state:
                    state[("h2", p)] = h2p.tile([128, S], bf16,
                                                tag="h2", name=f"h2_{p}")
                h2 = state[("h2", p)]
                pss = {j: c12ps.tile([128, CH], f32, tag="c12",
                                     name=f"c2_{p}_{j}") for j in js}
                for t9 in range(9):
                    kh, kw = divmod(t9, 3)
                    for j in js:
                        nc.tensor.matmul(
                            pss[j][:], w2sb[q][:, 128 * t9:128 * t9 + 128],
                            h1v[:, 8 * j + kh:8 * j + kh + 8, kw:kw + 56],
                            start=(t9 == 0), stop=(t9 == 8))
                for j in js:
                    nc.scalar.activation(h2[:, CH * j:CH * j + CH], pss[j][:],
                                         AF.Relu,
                                         bias=pk32a[:, BI + 5:BI + 6],
                                         scale=1.0)

            # Stage C(p) part 2: conv3 chunks of one (sample, out-half)
            # group; w3-stationary then identity-stationary runs.
            def stC_conv3(p, sl, h, js=range(NCH)):
                h2 = state[("h2", p)]
                xt = state[("xt", p)]
                po = 64 * sl
                key = ("ost", p, sl, h)
                if key not in state:
                    state[key] = outp.tile([128, S], bf16, tag="ost",
                                           name=f"ost_{p}_{sl}_{h}")
                ost = state[key]
                pss = {j: c3ps.tile([128, CH], f32, tag="c3",
                                    name=f"c3_{p}_{sl}_{h}_{j}") for j in js}
                for j in js:
                    nc.tensor.matmul(
                        pss[j][:],
                        pkbf[po:po + 64, W3 + 128 * h:W3 + 128 * h + 128],
                        h2[po:po + 64, CH * j:CH * j + CH],
                        start=True, stop=False)
                for j in js:
                    nc.tensor.matmul(
                        pss[j][:], pkbf[:, I1:I1 + 128],
                        xt[2 * sl + h][:, CH * j:CH * j + CH],
                        start=False, stop=True)
                for j in js:
                    if h == 1:
                        nc.scalar.activation(
                            ost[:, CH * j:CH * j + CH], pss[j][:], AF.Relu,
                            bias=pk32a[:, BI + 6 + h:BI + 7 + h], scale=1.0)
                    else:
                        nc.vector.tensor_scalar(
                            ost[:, CH * j:CH * j + CH], pss[j][:],
                            pk32a[:, BI + 6 + h:BI + 7 + h], 0.0,
                            op0=ALU.add, op1=ALU.max)
                if p == PAIRS - 1:
                    lo = CH * (js[0] - js[0] % 2)
                    hi = CH * (js[-1] + 1)
                    nc.sync.dma_start(
                        out_d[2 * p + sl, 128 * h:128 * h + 128, lo:hi],
                        ost[:, lo:hi])
                elif js[-1] == NCH - 1:
                    nc.sync.dma_start(
                        out_d[2 * p + sl, 128 * h:128 * h + 128, :], ost[:])

            # ================= pipelined emission =================
            def c3g(p, sl, h, part):
                js = ((0, 1), (2, 3), (4, 5), (6,))[part]
                stC_conv3(p, sl, h, js)

            # pair 0 prologue: per-sample routing so sample A's weights
            # are ready while sample B's x tiles are still arriving, then
            # conv1 with the A-channel half of every chunk first.
            p1_0 = sp.tile([128, 4], f32, tag="p1", name="p1_0")
            state[("p1", 0)] = p1_0
            p1b_0 = sp.tile([128, 4], bf16, tag="p1b", name="p1b_0")
            state[("p1b", 0)] = p1b_0
            for sl in range(2):
                stA_pool(0, 2 * sl, "dve")
                stA_pool(0, 2 * sl + 1, "act")
                t1sb = []
                for h in range(2):
                    tps = rps.tile([128, 1], f32, tag="rps")
                    for c in range(2):
                        nc.tensor.matmul(
                            tps[:],
                            pkr[:, R1B + 256 * c + 128 * h:
                                R1B + 256 * c + 128 * h + 128],
                            p1b_0[:, sl + 2 * c:sl + 2 * c + 1],
                            start=(c == 0), stop=(c == 1))
                    t = sp.tile([128, 1], f32, tag=f"t1sb{h}",
                                name=f"t1s0_{sl}_{h}")
                    nc.scalar.activation(t[:], tps[:], AF.Sigmoid,
                                         bias=pk32a[:, BI + h:BI + h + 1],
                                         scale=1.0)
                    t1sb.append(t)
                r1ps = rps.tile([4, 1], f32, tag="rps")
                for h in range(2):
                    nc.tensor.matmul(r1ps[:],
                                     pk32a[:, G + 4 * h:G + 4 * h + 4],
                                     t1sb[h][:], start=(h == 0), stop=(h == 1))
                r1sb = sp.tile([4, 1], f32, tag="r1sb",
                               name=f"r1s0_{sl}")
                nc.vector.tensor_copy(r1sb[:], r1ps[:])
                diag = sp.tile([4, 4], f32, tag="diag",
                               name=f"diag0_{sl}")
                nc.vector.tensor_scalar(diag[:], EYE4, r1sb[:], None,
                                        op0=ALU.mult)
                rbp = rps.tile([128, 4], f32, tag="rps")
                nc.tensor.matmul(rbp[:], ONES4, diag[:], start=True, stop=True)
                if ("rbc", 0) not in state:
                    state[("rbc", 0)] = sp.tile([128, 8], f32,
                                                tag="rbc", name="rbc_0")
                nc.vector.tensor_copy(
                    state[("rbc", 0)][:, 4 * sl:4 * sl + 4], rbp[:])
                stA_w1(0, sl)
                if sl == 0:
                    # A-channel half of conv1 over all 7 chunks keeps the
                    # PE busy while sample B's tiles/routing catch up.
                    xt0 = state[("xt", 0)]
                    state[("acc1", 0)] = sp.tile([128, NCH], f32,
                                                 tag="acc1", name="acc1_0")
                    c1ps = []
                    for j in range(NCH):
                        pool = c3ps if j < 5 else c12ps
                        tag = "c3" if j < 5 else "c12"
                        c1ps.append(pool.tile([128, CH], f32, tag=tag,
                                              name=f"c1p0_{j}"))
                    for c in range(2):
                        for j in range(NCH):
                            nc.tensor.matmul(
                                c1ps[j][:], w1sb[0][:, 128 * c:128 * c + 128],
                                xt0[c][:, CH * j:CH * j + CH],
                                start=(c == 0), stop=False)
            # B-channel half + evictions
            xt0 = state[("xt", 0)]
            for c in range(2, 4):
                for j in range(NCH):
                    nc.tensor.matmul(
                        c1ps[j][:], w1sb[0][:, 128 * c:128 * c + 128],
                        xt0[c][:, CH * j:CH * j + CH],
                        start=False, stop=(c == 3))
            for j in range(NCH):
                _c1_evict(0, j, c1ps[j])
            # pair-1 pooling rides the idle Pool engine so DVE/ACT stay
            # clear for pair-0's routing chain and evictions
            stA_pool(1, 0, "dve")
            stA_pool(1, 1, "act")

            for p in range(PAIRS):
                prv = p - 1 >= 0
                nxt = p + 1 < PAIRS
                last = p == PAIRS - 1

                def c3(sl, h, part):
                    if prv:
                        c3g(p - 1, sl, h, part)

                # -- window 1 -- conv1 first: its evictions feed the
                # routing-2 chain whose latency otherwise stalls conv2.
                # Pools for p+1 go after conv1 so they don't delay acc1.
                if p > 0:
                    if p >= 2:
                        stB_conv1(p, (0, 1))
                        c3(0, 0, 0)
                        stB_conv1(p, (2, 3))
                        c3(0, 0, 1)
                        stB_conv1(p, (4, 5))
                        c3(0, 0, 2)
                        stB_conv1(p, (6,))
                        c3(0, 0, 3)
                    else:
                        c3(0, 0, 0)
                        c3(0, 0, 1)
                        c3(0, 0, 2)
                        c3(0, 0, 3)
                    c3(0, 1, 0)
                    if nxt:
                        stA_pool(p + 1, 0, "dve")
                    c3(0, 1, 1)
                    c3(0, 1, 2)
                    if nxt:
                        stA_pool(p + 1, 1, "act")
                    c3(0, 1, 3)
                # -- window 2 -- routing-2 chain covered by c3 parts
                stB_pool2(p)
                c3(1, 0, 0)
                stB_r2(p)
                c3(1, 0, 1)
                stB_rl(p)
                c3(1, 0, 2)
                stB_w2(p)
                c3(1, 0, 3)
                if p == 0:
                    stA_pool(1, 2, "dve")
                    stA_pool(1, 3, "act")
                    stA_route(1)
                    stA_w1(1, 0)
                    stA_w1(1, 1)
                c3(1, 1, 0)
                c3(1, 1, 1)
                # -- window 3 -- conv2; for the last pair its conv3
                # starts as soon as h2 chunks land so the output DMA
                # overlaps compute instead of tailing after it.
                stC_conv2(p, (0, 1))
                if p == 0:
                    stB_conv1(1, (0, 1))
                elif nxt:
                    stA_pool(p + 1, 2, "act")
                stC_conv2(p, (2, 3))
                c3(1, 1, 2)
                if p == 0:
                    stB_conv1(1, (2, 3))
                elif nxt:
                    stA_pool(p + 1, 3, "dve")
                if last:
                    c3g(p, 0, 0, 0)
                stC_conv2(p, (4, 5))
                c3(1, 1, 3)
                if p == 0:
                    stB_conv1(1, (4, 5))
                elif nxt:
                    stA_route(p + 1)
                if last:
                    c3g(p, 0, 1, 0)
                stC_conv2(p, (6,))
                if p == 0:
                    stB_conv1(1, (6,))
                elif nxt:
                    stA_w1(p + 1, 0)
                    stA_w1(p + 1, 1)
                if last:
                    c3g(p, 0, 0, 1)
                    c3g(p, 0, 1, 1)
            # epilogue: remaining last-pair conv3, interleaved two
            # groups at a time so output DMA chunks stream out early
            lp = PAIRS - 1
            for part in (2, 3):
                c3g(lp, 0, 0, part)
                c3g(lp, 0, 1, part)
            for part in range(4):
                c3g(lp, 1, 0, part)
                c3g(lp, 1, 1, part)

    nc.compile()
    return nc


def _prep_consts(r1_W, r1_b, ew1, bn1_g, bn1_b, bn1_m, bn1_v,
                 r2_W, r2_b, ew2, bn2_g, bn2_b, bn2_m, bn2_v,
                 w3, bn3_g, bn3_b, bn3_m, bn3_v):
    f = np.float32
    s1 = (bn1_g / np.sqrt(bn1_v + EPS)).astype(f)
    b1 = (bn1_b - bn1_m * s1).astype(f)
    s2 = (bn2_g / np.sqrt(bn2_v + EPS)).astype(f)
    b2 = (bn2_b - bn2_m * s2).astype(f)
    s3 = (bn3_g / np.sqrt(bn3_v + EPS)).astype(f)
    b3 = (bn3_b - bn3_m * s3).astype(f)

    # ew1c [e, i128, (chunk, o)]  (bn1 scale folded)
    ew1s = ew1.reshape(E, WD, C) * s1[None, :, None]          # [e, o, i]
    ew1c = np.ascontiguousarray(
        ew1s.transpose(0, 2, 1)                                # [e, i, o]
        .reshape(E, 2, 128, WD)                                # [e, c, i128, o]
        .transpose(0, 2, 1, 3)                                 # [e, i128, c, o]
        .reshape(E, 128, 128)).astype(f)

    # ew2c [chunk, (e2, i), (tap, o)]  (bn2 scale folded)
    ew2s = ew2.reshape(E, WD, WD, 9) * s2[None, :, None, None]  # [e, o, i, t]
    ew2c = np.ascontiguousarray(
        ew2s.transpose(0, 2, 3, 1)                             # [e, i, t, o]
        .reshape(2, 128, 9 * WD))

    w3h = (w3 * s3[:, None]).T.astype(np.float32)              # [i 64, o 256]
    w3t = np.concatenate([w3h, w3h], 0)

    i128 = np.eye(128, dtype=f)
    r1wt = np.ascontiguousarray((r1_W.T / S).reshape(2, 128, D)).astype(f)
    r2h = (r2_W.T / S).astype(f)                               # [64, 256]
    r2wt = np.concatenate([r2h, r2h], 0)

    g = np.zeros((D, E), f)
    g[np.arange(D), np.arange(D) // WD] = 1.0 / WD
    gsel = np.ascontiguousarray(g.reshape(2, 128, E))

    sm4 = np.zeros((4, 388), f)
    sm4[:, 0:4] = np.eye(4, dtype=f)
    sm4[:, 4:132] = 1.0
    sm4[0, 132:196] = 1.0
    sm4[1, 196:260] = 1.0
    sm4[2, 260:324] = 1.0
    sm4[3, 324:388] = 1.0

    eye2 = np.concatenate([np.eye(WD, dtype=f), np.eye(WD, dtype=f)], 0)

    bias = np.zeros((128, 8), f)
    bias[:, 0] = r1_b[0:128]
    bias[:, 1] = r1_b[128:256]
    bias[:, 2] = r2_b[0:128]
    bias[:, 3] = r2_b[128:256]
    bias[:, 4] = np.concatenate([b1, b1])
    bias[:, 5] = np.concatenate([b2, b2])
    bias[:, 6] = b3[0:128]
    bias[:, 7] = b3[128:256]

    bf = ml_dtypes.bfloat16
    pk32a = np.ascontiguousarray(np.concatenate(
        [r1wt[0], r1wt[1], gsel[0], gsel[1], bias,
         ew1c[0], ew1c[1], ew1c[2], ew1c[3]], axis=1)).astype(f)
    pk32b = np.ascontiguousarray(np.concatenate([r2wt, eye2], axis=1)).astype(f)
    pkr = np.ascontiguousarray(np.concatenate(
        [r1wt[0], r1wt[1]], axis=1)).astype(ml_dtypes.bfloat16)
    pkbf = np.ascontiguousarray(np.concatenate(
        [ew2c[0], ew2c[1], w3t, i128], axis=1)).astype(bf)

    return dict(pk32a=pk32a, pk32b=pk32b, pkr=pkr, pkbf=pkbf, sm4=sm4)


def kernel(x, **weights):
    if "nc" not in _cache:
        _cache["nc"] = _build()
    nc = _cache["nc"]
    consts = _prep_consts(**{k: np.asarray(v) for k, v in weights.items()})
    xf = np.asarray(x, dtype=np.float32).reshape(B, C, S).astype(
        ml_dtypes.bfloat16)
    in_maps = []
    for c in range(N_CORES):
        m = {"x": np.ascontiguousarray(xf[BPC * c:BPC * (c + 1)])}
        m.update(consts)
        in_maps.append(m)
    res = run_bass_kernel_spmd(nc, in_maps, core_ids=list(range(N_CORES)),
                               **_cache.get("run_kwargs", {}))
    _cache["last_res"] = res
    out = np.concatenate(
        [res.results[c]["out"][None] for c in range(N_CORES)], 0)
    return out.astype(np.float32).reshape(B, C, HW, HW)



# revision 11
# speedup vs baseline: 1.0408x; 1.0252x over previous
"""Trainium2 Bass kernel for nn_MoEBottleneck (moe_routing).

Data-parallel over batch: 64 samples sharded 8-per-core across 8 NeuronCores.
Per core, samples are processed in pairs packed onto the 128 SBUF partitions.

Computation per sample (C=256 in/out channels, width=64, 56x56 spatial, E=4):
  r1 = groupmean(sigmoid(r1_W @ mean_hw(x) + r1_b))          routing 1
  h1 = relu(bn1(combine(r1, ew1) @ x))                       1x1 CondConv
  r2 = groupmean(sigmoid(r2_W @ mean_hw(h1) + r2_b))         routing 2
  h2 = relu(bn2(conv3x3(combine(r2, ew2), h1)))              3x3 CondConv
  out = relu(bn3(w3 @ h2) + x)                               1x1 + residual

x is cast to bf16 on the host and the output is written bf16 and upcast on
the host, halving both directions of HBM traffic.  BN scales are folded into
the expert weights host-side; BN biases ride the per-partition bias port of
the eviction ops.  The residual add is folded into conv3 as an identity-
weight matmul accumulating into the same PSUM bank.  Routing stays fp32.

Scheduling notes:
- Weight-stationary inner loops (one LDWEIGHTS per stationary covering a
  group of spatial chunks) keep the PE at its ~190ns/448-col issue rate.
- Constants ride 3 packed DMAs; DMA descriptor issues are throttled by a
  global 8-deep completion-semaphore rotation, so descriptor count is kept
  low and issue order matches need order.
- Pooled means are split between DVE column-reduces and in-place ACT
  copies with the fp32 accumulator sidecar, so no single queue
  serializes the routing chain (Pool-engine streaming ops turned out to
  stall concurrent DVE ops and are avoided).
- Pair-1's conv1 is interleaved into pair-0's conv2 window to shorten the
  pipeline fill; pair-0's conv1 runs per-sample halves as its x tiles land.
"""

import sys

for _p in ("/opt/trn_rl_repo",):
    if _p not in sys.path:
        sys.path.insert(0, _p)

import ml_dtypes
import numpy as np

import concourse.bass as bass
import concourse.tile as tile
from concourse import bacc, mybir
from concourse.bass_utils import run_bass_kernel_spmd

dt = mybir.dt
AF = mybir.ActivationFunctionType
ALU = mybir.AluOpType

N_CORES = 8
B, C, HW, S = 64, 256, 56, 56 * 56          # batch, channels, spatial
WD, E, D = 64, 4, 256                        # width, experts, routing interm
BPC = B // N_CORES                           # samples per core (8)
PAIRS = BPC // 2
EPS = 1e-5
NCH = 7                                      # spatial chunks (8 rows x 56 = 448)
CH = S // NCH                                # 448
PW = HW + 2                                  # padded row width 58

# packed-constant column offsets (see _prep_consts)
R1, G, BI, E1 = 0, 512, 520, 528             # in pk32a [128, 1040] f32
R2, EY = 0, 256                              # in pk32b [128, 320]  f32
EW2, W3, I1 = 0, 1152, 1408                  # in pkbf  [128, 1536] bf16
R1B = 0                                      # in pkr   [128, 512]  bf16

_cache = {}


def _build():
    nc = bacc.Bacc("TRN2", target_bir_lowering=False, debug=False,
                   num_devices=N_CORES)
    f32, bf16 = dt.float32, dt.bfloat16

    x_d = nc.dram_tensor("x", [BPC, C, S], bf16, kind="ExternalInput").ap()
    pk32a_d = nc.dram_tensor("pk32a", [128, 1040], f32, kind="ExternalInput").ap()
    pk32b_d = nc.dram_tensor("pk32b", [128, 320], f32, kind="ExternalInput").ap()
    pkr_d = nc.dram_tensor("pkr", [128, 512], bf16, kind="ExternalInput").ap()
    pkbf_d = nc.dram_tensor("pkbf", [128, 1536], bf16, kind="ExternalInput").ap()
    sm4_d = nc.dram_tensor("sm4", [4, 388], f32, kind="ExternalInput").ap()
    out_d = nc.dram_tensor("out", [BPC, C, S], bf16, kind="ExternalOutput").ap()

    with tile.TileContext(nc) as tc:
        with tc.tile_pool(name="const", bufs=1) as cp, \
             tc.tile_pool(name="pers", bufs=1) as pp, \
             tc.tile_pool(name="xp", bufs=16) as xp, \
             tc.tile_pool(name="h2p", bufs=2) as h2p, \
             tc.tile_pool(name="outp", bufs=4) as outp, \
             tc.tile_pool(name="small", bufs=2) as sp, \
             tc.tile_pool(name="c12ps", bufs=2, space="PSUM") as c12ps, \
             tc.tile_pool(name="c3ps", bufs=5, space="PSUM") as c3ps, \
             tc.tile_pool(name="rps", bufs=1, space="PSUM") as rps:

            state = {}

            def stA_dma(p, ks=range(4)):
                sa, sb = 2 * p, 2 * p + 1
                locs = ((sa, 0), (sa, 1), (sb, 0), (sb, 1))
                xt = state.setdefault(("xt", p), [None] * 4)
                for k in ks:
                    s, h = locs[k]
                    t = xp.tile([128, S], bf16, tag="xt", name=f"xt_{p}_{k}")
                    nc.sync.dma_start(t[:], x_d[s, 128 * h:128 * h + 128, :])
                    xt[k] = t

            # ---- persistent-buffer memsets first (gpsimd queue) ----
            h1p, w1sb, w2sb = [], [], []
            for q in range(2):
                h1p.append(pp.tile([128, PW * PW], bf16, tag=f"h1p{q}",
                                   name=f"h1p{q}"))
                w1sb.append(pp.tile([128, 512], bf16, tag=f"w1sb{q}",
                                    name=f"w1sb{q}"))
                w2sb.append(pp.tile([128, 1152], bf16, tag=f"w2sb{q}",
                                    name=f"w2sb{q}"))
            for q in range(2):
                tv = w1sb[q][:].rearrange("p (c m) -> p c m", m=128)
                nc.gpsimd.memset(tv[:, 0:2, 64:128], 0.0)
                nc.gpsimd.memset(tv[:, 2:4, 0:64], 0.0)
                tv = h1p[q][:].rearrange("p (r c) -> p r c", r=PW)
                nc.gpsimd.memset(tv[:, 0:1, :], 0.0)
                nc.gpsimd.memset(tv[:, PW - 1:PW, :], 0.0)
                nc.gpsimd.memset(tv[:, :, 0:1], 0.0)
                nc.gpsimd.memset(tv[:, :, PW - 1:PW], 0.0)
                tv = w2sb[q][:].rearrange("p (t m) -> p t m", m=128)
                nc.gpsimd.memset(tv[0:64, :, 64:128], 0.0)
                nc.gpsimd.memset(tv[64:128, :, 0:64], 0.0)

            # ---- DMAs in need-order on the sync queue; all 16 x tiles
            # are prefetched upfront (SBUF holds them all) so no mid-
            # stream DMA waits gate later pairs ----
            stA_dma(0, (0, 1))                      # sample-A tiles
            pk32a = cp.tile([128, 1040], f32, tag="pk32a")
            nc.sync.dma_start(pk32a[:], pk32a_d)    # routing+w1 consts
            pkr = cp.tile([128, 512], bf16, tag="pkr")
            nc.sync.dma_start(pkr[:], pkr_d)        # bf16 r1 weights
            sm4 = cp.tile([4, 388], f32, tag="sm4")
            nc.sync.dma_start(sm4[:], sm4_d[:])     # routing group masks
            stA_dma(0, (2, 3))                      # sample-B tiles
            stA_dma(1, (0, 1))                      # pair-1 sample-A
            pk32b = cp.tile([128, 320], f32, tag="pk32b")
            nc.sync.dma_start(pk32b[:], pk32b_d)    # routing2 consts
            pkbf = cp.tile([128, 1536], bf16, tag="pkbf")
            nc.sync.dma_start(pkbf[:], pkbf_d)      # conv2/conv3 weights
            stA_dma(1, (2, 3))                      # pair-1 sample-B
            stA_dma(2, (0, 1))
            stA_dma(2, (2, 3))
            stA_dma(3, (0, 1))
            stA_dma(3, (2, 3))
            EYE4 = sm4[:, 0:4]
            ONES4 = sm4[:, 4:132]
            E01 = sm4[:, 132:260]
            E23 = sm4[:, 260:388]

            # ================= stage emitters =================
            # Pooled column-sums, chunked into S/4-wide pieces so no
            # single long op blocks an engine FIFO; the last chunk rides
            # the ACT accumulator of an in-place copy.  The final small
            # reduce writes the bf16 routing operand p1b directly.
            QC = S // 4                              # 784-col pool chunks

            def stA_pool(p, k, mode=None):
                if ("p1b", p) not in state:
                    state[("p1b", p)] = sp.tile([128, 4], bf16, tag="p1b",
                                                name=f"p1b_{p}")
                p1b = state[("p1b", p)]
                xt = state[("xt", p)]
                col = (0, 2, 1, 3)[k]
                pacc = sp.tile([128, 4], f32, tag=f"pacc{k}",
                               name=f"pacc_{p}_{k}")
                for j in range(3):
                    nc.vector.tensor_reduce(pacc[:, j:j + 1],
                                            xt[k][:, QC * j:QC * j + QC],
                                            axis=mybir.AxisListType.X,
                                            op=ALU.add)
                nc.scalar.activation(xt[k][:, 3 * QC:S], xt[k][:, 3 * QC:S],
                                     AF.Copy, accum_out=pacc[:, 3:4])
                p1c = sp.tile([128, 1], f32, tag=f"p1c{k}",
                              name=f"p1c_{p}_{k}")
                nc.vector.tensor_reduce(p1c[:], pacc[:],
                                        axis=mybir.AxisListType.X, op=ALU.add)
                nc.vector.tensor_copy(p1b[:, col:col + 1], p1c[:])

            def stA_route(p):
                p1b = state[("p1b", p)]
                t1sb = []
                for h in range(2):
                    tps = rps.tile([128, 2], f32, tag="rps")
                    for c in range(2):
                        nc.tensor.matmul(
                            tps[:],
                            pkr[:, R1B + 256 * c + 128 * h:
                                R1B + 256 * c + 128 * h + 128],
                            p1b[:, 2 * c:2 * c + 2],
                            start=(c == 0), stop=(c == 1))
                    t = sp.tile([128, 2], f32, tag=f"t1sb{h}")
                    nc.scalar.activation(t[:], tps[:], AF.Sigmoid,
                                         bias=pk32a[:, BI + h:BI + h + 1],
                                         scale=1.0)
                    t1sb.append(t)
                r1ps = rps.tile([4, 2], f32, tag="rps")
                for h in range(2):
                    nc.tensor.matmul(r1ps[:],
                                     pk32a[:, G + 4 * h:G + 4 * h + 4],
                                     t1sb[h][:], start=(h == 0), stop=(h == 1))
                r1sb = sp.tile([4, 2], f32, tag="r1sb")
                nc.vector.tensor_copy(r1sb[:], r1ps[:])
                diag = sp.tile([4, 8], f32, tag="diag")
                for sl in range(2):
                    nc.vector.tensor_scalar(diag[:, 4 * sl:4 * sl + 4], EYE4,
                                            r1sb[:, sl:sl + 1], None,
                                            op0=ALU.mult)
                rbp = rps.tile([128, 8], f32, tag="rps")
                nc.tensor.matmul(rbp[:], ONES4, diag[:], start=True, stop=True)
                rbc = sp.tile([128, 8], f32, tag="rbc")
                nc.vector.tensor_copy(rbc[:], rbp[:])
                state[("rbc", p)] = rbc

            def stA_w1(p, sl):
                rbc = state[("rbc", p)]
                scr = sp.tile([128, 128], f32, tag="w1scr",
                              name=f"w1scr_{p}_{sl}")
                for e in range(E):
                    if e == 0:
                        nc.vector.tensor_scalar(
                            scr[:], pk32a[:, E1:E1 + 128],
                            rbc[:, 4 * sl:4 * sl + 1],
                            None, op0=ALU.mult)
                    else:
                        nc.vector.scalar_tensor_tensor(
                            scr[:], pk32a[:, E1 + 128 * e:E1 + 128 * e + 128],
                            rbc[:, 4 * sl + e:4 * sl + e + 1], scr[:],
                            op0=ALU.mult, op1=ALU.add)
                w1v = w1sb[p % 2][:].rearrange("p (c m) -> p c m", m=128)
                dst = w1v[:, 2 * sl:2 * sl + 2, 64 * sl:64 * sl + 64]
                nc.scalar.copy(dst, scr[:].rearrange("p (c o) -> p c o", o=64))

            # Stage B(p): conv1 + bn1 + pooled2 + routing2 + w2.
            # Weight-stationary: one LDWEIGHTS per contraction chunk for
            # all of js; evictions alternate ACT/DVE (both carry the
            # pooled-h1 accumulator).
            def _c1_evict(p, j, ps):
                q = p % 2
                h1v = h1p[q][:].rearrange("p (r c) -> p r c", r=PW)
                acc1 = state[("acc1", p)]
                dstv = h1v[:, 1 + 8 * j:9 + 8 * j, 1:57]
                nc.scalar.activation(
                    dstv, ps[:], AF.Relu, bias=pk32a[:, BI + 4:BI + 5],
                    scale=1.0, accum_out=acc1[:, j:j + 1])

            def stB_conv1(p, js):
                q = p % 2
                xt = state[("xt", p)]
                if ("acc1", p) not in state:
                    state[("acc1", p)] = sp.tile([128, NCH], f32,
                                                 tag="acc1", name=f"acc1_{p}")
                pss = {j: c12ps.tile([128, CH], f32, tag="c12",
                                     name=f"c1_{p}_{j}") for j in js}
                for c in range(4):
                    for j in js:
                        nc.tensor.matmul(
                            pss[j][:], w1sb[q][:, 128 * c:128 * c + 128],
                            xt[c][:, CH * j:CH * j + CH],
                            start=(c == 0), stop=(c == 3))
                for j in js:
                    _c1_evict(p, j, pss[j])

            def stB_pool2(p):
                acc1 = state[("acc1", p)]
                p2 = sp.tile([128, 1], f32, tag="p2")
                nc.vector.tensor_reduce(p2[:], acc1[:],
                                        axis=mybir.AxisListType.X, op=ALU.add)
                t2sb = []
                for h in range(2):
                    tps = rps.tile([128, 2], f32, tag="rps")
                    for sl in range(2):
                        po = 64 * sl
                        nc.tensor.matmul(
                            tps[:, sl:sl + 1],
                            pk32b[po:po + 64, R2 + 128 * h:R2 + 128 * h + 128],
                            p2[po:po + 64, :], start=True, stop=True)
                    t = sp.tile([128, 2], f32, tag=f"t2sb{h}")
                    nc.scalar.activation(t[:], tps[:], AF.Sigmoid,
                                         bias=pk32a[:, BI + 2 + h:BI + 3 + h],
                                         scale=1.0)
                    t2sb.append(t)
                state[("t2sb", p)] = t2sb

            def stB_r2(p):
                t2sb = state[("t2sb", p)]
                r2ps = rps.tile([4, 2], f32, tag="rps")
                for h in range(2):
                    nc.tensor.matmul(r2ps[:],
                                     pk32a[:, G + 4 * h:G + 4 * h + 4],
                                     t2sb[h][:], start=(h == 0), stop=(h == 1))
                r2sb = sp.tile([4, 2], f32, tag="r2sb")
                nc.vector.tensor_copy(r2sb[:], r2ps[:])
                cols = []
                for c, sel in enumerate((E01, E23)):
                    cps = rps.tile([128, 2], f32, tag="rps")
                    nc.tensor.matmul(cps[:], sel, r2sb[:], start=True, stop=True)
                    t = sp.tile([128, 2], f32, tag=f"cols{c}")
                    nc.vector.tensor_copy(t[:], cps[:])
                    cols.append(t)
                state[("cols", p)] = cols

            def stB_rl(p):
                cols = state[("cols", p)]
                rl = sp.tile([128, 256], bf16, tag="rl")
                for c in range(2):
                    nc.vector.tensor_scalar(
                        rl[:, 128 * c:128 * c + 64], pk32b[:, EY:EY + 64],
                        cols[c][:, 0:1], None, op0=ALU.mult)
                    nc.vector.tensor_scalar(
                        rl[:, 128 * c + 64:128 * c + 128],
                        pk32b[:, EY:EY + 64],
                        cols[c][:, 1:2], None, op0=ALU.mult)
                state[("rl", p)] = rl

            def stB_w2(p):
                q = p % 2
                rl = state[("rl", p)]
                w2v = w2sb[q][:].rearrange("p (t m) -> p t m", m=128)
                for g0, g1 in ((0, 512), (512, 576)):
                    wps = rps.tile([128, g1 - g0], f32, tag="rps")
                    for c in range(2):
                        nc.tensor.matmul(
                            wps[:], rl[:, 128 * c:128 * c + 128],
                            pkbf[:, EW2 + 576 * c + g0:EW2 + 576 * c + g1],
                            start=(c == 0), stop=(c == 1))
                    wpv = wps[:].rearrange("p (t o) -> p t o", o=64)
                    t0, t1 = g0 // 64, g1 // 64
                    nc.vector.tensor_copy(w2v[0:64, t0:t1, 0:64], wpv[0:64])
                    nc.vector.tensor_copy(w2v[64:128, t0:t1, 64:128], wpv[64:128])

            # Stage C(p) part 1: conv2 + bn2 -> h2 (tap-stationary).
            def stC_conv2(p, js):
                q = p % 2
                h1v = h1p[q][:].rearrange("p (r c) -> p r c", r=PW)
                if ("h2", p) not in state:
                    state[("h2", p)] = h2p.tile([128, S], bf16,
                                                tag="h2", name=f"h2_{p}")
                h2 = state[("h2", p)]
                pss = {j: c12ps.tile([128, CH], f32, tag="c12",
                                     name=f"c2_{p}_{j}") for j in js}
                for t9 in range(9):
                    kh, kw = divmod(t9, 3)
                    for j in js:
                        nc.tensor.matmul(
                            pss[j][:], w2sb[q][:, 128 * t9:128 * t9 + 128],
                            h1v[:, 8 * j + kh:8 * j + kh + 8, kw:kw + 56],
                            start=(t9 == 0), stop=(t9 == 8))
                for j in js:
                    nc.scalar.activation(h2[:, CH * j:CH * j + CH], pss[j][:],
                                         AF.Relu,
                                         bias=pk32a[:, BI + 5:BI + 6],
                                         scale=1.0)

            # Stage C(p) part 2: conv3 chunks of one (sample, out-half)
            # group; w3-stationary then identity-stationary runs.
            def stC_conv3(p, sl, h, js=range(NCH)):
                h2 = state[("h2", p)]
                xt = state[("xt", p)]
                po = 64 * sl
                key = ("ost", p, sl, h)
                if key not in state:
                    state[key] = outp.tile([128, S], bf16, tag="ost",
                                           name=f"ost_{p}_{sl}_{h}")
                ost = state[key]
                pss = {j: c3ps.tile([128, CH], f32, tag="c3",
                                    name=f"c3_{p}_{sl}_{h}_{j}") for j in js}
                for j in js:
                    nc.tensor.matmul(
                        pss[j][:],
                        pkbf[po:po + 64, W3 + 128 * h:W3 + 128 * h + 128],
                        h2[po:po + 64, CH * j:CH * j + CH],
                        start=True, stop=False)
                for j in js:
                    nc.tensor.matmul(
                        pss[j][:], pkbf[:, I1:I1 + 128],
                        xt[2 * sl + h][:, CH * j:CH * j + CH],
                        start=False, stop=True)
                for j in js:
                    if h == 1:
                        nc.scalar.activation(
                            ost[:, CH * j:CH * j + CH], pss[j][:], AF.Relu,
                            bias=pk32a[:, BI + 6 + h:BI + 7 + h], scale=1.0)
                    else:
                        nc.vector.tensor_scalar(
                            ost[:, CH * j:CH * j + CH], pss[j][:],
                            pk32a[:, BI + 6 + h:BI + 7 + h], 0.0,
                            op0=ALU.add, op1=ALU.max)
                if p == PAIRS - 1:
                    lo = CH * (js[0] - js[0] % 2)
                    hi = CH * (js[-1] + 1)
                    nc.sync.dma_start(
                        out_d[2 * p + sl, 128 * h:128 * h + 128, lo:hi],
                        ost[:, lo:hi])
                elif js[-1] == NCH - 1:
                    nc.sync.dma_start(
                        out_d[2 * p + sl, 128 * h:128 * h + 128, :], ost[:])

            # ================= pipelined emission =================
            def c3g(p, sl, h, part):
                js = ((0, 1), (2, 3), (4, 5), (6,))[part]
                stC_conv3(p, sl, h, js)

            # Single-sample routing chain (cols sl, sl+2 of p1b) — used
            # at startup so a sample's conv1 can begin while later x
            # tiles are still in flight.
            def route_sample(p, sl):
                p1b = state[("p1b", p)]
                t1sb = []
                for h in range(2):
                    tps = rps.tile([128, 1], f32, tag="rps")
                    for c in range(2):
                        nc.tensor.matmul(
                            tps[:],
                            pkr[:, R1B + 256 * c + 128 * h:
                                R1B + 256 * c + 128 * h + 128],
                            p1b[:, sl + 2 * c:sl + 2 * c + 1],
                            start=(c == 0), stop=(c == 1))
                    t = sp.tile([128, 1], f32, tag=f"t1sb{h}",
                                name=f"t1s_{p}_{sl}_{h}")
                    nc.scalar.activation(t[:], tps[:], AF.Sigmoid,
                                         bias=pk32a[:, BI + h:BI + h + 1],
                                         scale=1.0)
                    t1sb.append(t)
                r1ps = rps.tile([4, 1], f32, tag="rps")
                for h in range(2):
                    nc.tensor.matmul(r1ps[:],
                                     pk32a[:, G + 4 * h:G + 4 * h + 4],
                                     t1sb[h][:], start=(h == 0), stop=(h == 1))
                r1sb = sp.tile([4, 1], f32, tag="r1sb",
                               name=f"r1s_{p}_{sl}")
                nc.vector.tensor_copy(r1sb[:], r1ps[:])
                diag = sp.tile([4, 4], f32, tag="diag",
                               name=f"diag_{p}_{sl}")
                nc.vector.tensor_scalar(diag[:], EYE4, r1sb[:], None,
                                        op0=ALU.mult)
                rbp = rps.tile([128, 4], f32, tag="rps")
                nc.tensor.matmul(rbp[:], ONES4, diag[:], start=True, stop=True)
                if ("rbc", p) not in state:
                    state[("rbc", p)] = sp.tile([128, 8], f32,
                                                tag="rbc", name=f"rbc_{p}")
                nc.vector.tensor_copy(
                    state[("rbc", p)][:, 4 * sl:4 * sl + 4], rbp[:])

            # Partial conv1: accumulate channel-chunks cs for spatial js
            # into the pair's held PSUM tiles.
            def conv1_mm(p, js, cs, start, stop):
                xt = state[("xt", p)]
                pss = state[("c1ps", p)]
                q = p % 2
                for c in cs:
                    for j in js:
                        nc.tensor.matmul(
                            pss[j][:], w1sb[q][:, 128 * c:128 * c + 128],
                            xt[c][:, CH * j:CH * j + CH],
                            start=(start and c == cs[0]),
                            stop=(stop and c == cs[-1]))

            # pair-0 prologue: per-sample routing, conv1 A-half first.
            for sl in range(2):
                stA_pool(0, 2 * sl)
                stA_pool(0, 2 * sl + 1)
                route_sample(0, sl)
                stA_w1(0, sl)
                if sl == 0:
                    state[("acc1", 0)] = sp.tile([128, NCH], f32,
                                                 tag="acc1", name="acc1_0")
                    c1ps = []
                    for j in range(NCH):
                        pool = c3ps if j < 5 else c12ps
                        tag = "c3" if j < 5 else "c12"
                        c1ps.append(pool.tile([128, CH], f32, tag=tag,
                                              name=f"c1p0_{j}"))
                    state[("c1ps", 0)] = c1ps
                    conv1_mm(0, range(NCH), (0, 1), True, False)
            conv1_mm(0, range(NCH), (2, 3), False, True)
            for j in range(NCH):
                _c1_evict(0, j, c1ps[j])
            # pair-1 sample-A: routing + A-half conv1 of chunks 0-3
            # bridges the PE gap before pair-0's conv2 weights exist.
            stA_pool(1, 0)
            stA_pool(1, 1)
            route_sample(1, 0)
            stA_w1(1, 0)
            state[("acc1", 1)] = sp.tile([128, NCH], f32,
                                         tag="acc1", name="acc1_1")
            c1ps1 = [c3ps.tile([128, CH], f32, tag="c3", name=f"c1p1_{j}")
                     for j in range(4)]
            state[("c1ps", 1)] = c1ps1
            conv1_mm(1, range(4), (0, 1), True, False)

            for p in range(PAIRS):
                prv = p - 1 >= 0
                nxt = p + 1 < PAIRS
                last = p == PAIRS - 1

                def c3(sl, h, part):
                    if prv:
                        c3g(p - 1, sl, h, part)

                # -- window 1 -- conv1 first: its evictions feed the
                # routing-2 chain whose latency otherwise stalls conv2.
                # Pools for p+1 go after conv1 so they don't delay acc1.
                if p > 0:
                    if p >= 2:
                        stB_conv1(p, (0, 1))
                        c3(0, 0, 0)
                        stB_conv1(p, (2, 3))
                        c3(0, 0, 1)
                        stB_conv1(p, (4, 5))
                        c3(0, 0, 2)
                        stB_conv1(p, (6,))
                        c3(0, 0, 3)
                    else:
                        c3(0, 0, 0)
                        c3(0, 0, 1)
                        c3(0, 0, 2)
                        c3(0, 0, 3)
                    c3(0, 1, 0)
                    if nxt:
                        stA_pool(p + 1, 0)
                    c3(0, 1, 1)
                    c3(0, 1, 2)
                    if nxt:
                        stA_pool(p + 1, 1)
                    c3(0, 1, 3)
                # -- window 2 -- routing-2 chain covered by c3 parts
                stB_pool2(p)
                c3(1, 0, 0)
                stB_r2(p)
                c3(1, 0, 1)
                stB_rl(p)
                c3(1, 0, 2)
                stB_w2(p)
                c3(1, 0, 3)
                if p == 0:
                    stA_pool(1, 2)
                    stA_pool(1, 3)
                    route_sample(1, 1)
                    stA_w1(1, 1)
                c3(1, 1, 0)
                c3(1, 1, 1)
                # -- window 3 -- conv2; for the last pair its conv3
                # starts as soon as h2 chunks land so the output DMA
                # overlaps compute instead of tailing after it.
                stC_conv2(p, (0, 1))
                if p == 0:
                    conv1_mm(1, range(4), (2, 3), False, True)
                    for j in range(4):
                        _c1_evict(1, j, state[("c1ps", 1)][j])
                elif nxt:
                    stA_pool(p + 1, 2)
                stC_conv2(p, (2, 3))
                c3(1, 1, 2)
                if p == 0:
                    stB_conv1(1, (4, 5))
                elif nxt:
                    stA_pool(p + 1, 3)
                if last:
                    c3g(p, 0, 0, 0)
                stC_conv2(p, (4, 5))
                c3(1, 1, 3)
                if p == 0:
                    stB_conv1(1, (6,))
                elif nxt:
                    stA_route(p + 1)
                if last:
                    c3g(p, 0, 1, 0)
                stC_conv2(p, (6,))
                if (not last) and p > 0:
                    stA_w1(p + 1, 0)
                    stA_w1(p + 1, 1)
                if last:
                    c3g(p, 0, 0, 1)
                    c3g(p, 0, 1, 1)
            # epilogue: remaining last-pair conv3, interleaved two
            # groups at a time so output DMA chunks stream out early
            lp = PAIRS - 1
            for part in (2, 3):
                c3g(lp, 0, 0, part)
                c3g(lp, 0, 1, part)
            for part in range(4):
                c3g(lp, 1, 0, part)
                c3g(lp, 1, 1, part)

    nc.compile()
    return nc


def _prep_consts(r1_W, r1_b, ew1, bn1_g, bn1_b, bn1_m, bn1_v,
                 r2_W, r2_b, ew2, bn2_g, bn2_b, bn2_m, bn2_v,
                 w3, bn3_g, bn3_b, bn3_m, bn3_v):
    f = np.float32
    s1 = (bn1_g / np.sqrt(bn1_v + EPS)).astype(f)
    b1 = (bn1_b - bn1_m * s1).astype(f)
    s2 = (bn2_g / np.sqrt(bn2_v + EPS)).astype(f)
    b2 = (bn2_b - bn2_m * s2).astype(f)
    s3 = (bn3_g / np.sqrt(bn3_v + EPS)).astype(f)
    b3 = (bn3_b - bn3_m * s3).astype(f)

    # ew1c [e, i128, (chunk, o)]  (bn1 scale folded)
    ew1s = ew1.reshape(E, WD, C) * s1[None, :, None]          # [e, o, i]
    ew1c = np.ascontiguousarray(
        ew1s.transpose(0, 2, 1)                                # [e, i, o]
        .reshape(E, 2, 128, WD)                                # [e, c, i128, o]
        .transpose(0, 2, 1, 3)                                 # [e, i128, c, o]
        .reshape(E, 128, 128)).astype(f)

    # ew2c [chunk, (e2, i), (tap, o)]  (bn2 scale folded)
    ew2s = ew2.reshape(E, WD, WD, 9) * s2[None, :, None, None]  # [e, o, i, t]
    ew2c = np.ascontiguousarray(
        ew2s.transpose(0, 2, 3, 1)                             # [e, i, t, o]
        .reshape(2, 128, 9 * WD))

    w3h = (w3 * s3[:, None]).T.astype(np.float32)              # [i 64, o 256]
    w3t = np.concatenate([w3h, w3h], 0)

    i128 = np.eye(128, dtype=f)
    r1wt = np.ascontiguousarray((r1_W.T / S).reshape(2, 128, D)).astype(f)
    r2h = (r2_W.T / S).astype(f)                               # [64, 256]
    r2wt = np.concatenate([r2h, r2h], 0)

    g = np.zeros((D, E), f)
    g[np.arange(D), np.arange(D) // WD] = 1.0 / WD
    gsel = np.ascontiguousarray(g.reshape(2, 128, E))

    sm4 = np.zeros((4, 388), f)
    sm4[:, 0:4] = np.eye(4, dtype=f)
    sm4[:, 4:132] = 1.0
    sm4[0, 132:196] = 1.0
    sm4[1, 196:260] = 1.0
    sm4[2, 260:324] = 1.0
    sm4[3, 324:388] = 1.0

    eye2 = np.concatenate([np.eye(WD, dtype=f), np.eye(WD, dtype=f)], 0)

    bias = np.zeros((128, 8), f)
    bias[:, 0] = r1_b[0:128]
    bias[:, 1] = r1_b[128:256]
    bias[:, 2] = r2_b[0:128]
    bias[:, 3] = r2_b[128:256]
    bias[:, 4] = np.concatenate([b1, b1])
    bias[:, 5] = np.concatenate([b2, b2])
    bias[:, 6] = b3[0:128]
    bias[:, 7] = b3[128:256]

    bf = ml_dtypes.bfloat16
    pk32a = np.ascontiguousarray(np.concatenate(
        [r1wt[0], r1wt[1], gsel[0], gsel[1], bias,
         ew1c[0], ew1c[1], ew1c[2], ew1c[3]], axis=1)).astype(f)
    pk32b = np.ascontiguousarray(np.concatenate([r2wt, eye2], axis=1)).astype(f)
    pkr = np.ascontiguousarray(np.concatenate(
        [r1wt[0], r1wt[1]], axis=1)).astype(ml_dtypes.bfloat16)
    pkbf = np.ascontiguousarray(np.concatenate(
        [ew2c[0], ew2c[1], w3t, i128], axis=1)).astype(bf)

    return dict(pk32a=pk32a, pk32b=pk32b, pkr=pkr, pkbf=pkbf, sm4=sm4)


def kernel(x, **weights):
    if "nc" not in _cache:
        _cache["nc"] = _build()
    nc = _cache["nc"]
    consts = _prep_consts(**{k: np.asarray(v) for k, v in weights.items()})
    xf = np.asarray(x, dtype=np.float32).reshape(B, C, S).astype(
        ml_dtypes.bfloat16)
    in_maps = []
    for c in range(N_CORES):
        m = {"x": np.ascontiguousarray(xf[BPC * c:BPC * (c + 1)])}
        m.update(consts)
        in_maps.append(m)
    res = run_bass_kernel_spmd(nc, in_maps, core_ids=list(range(N_CORES)),
                               **_cache.get("run_kwargs", {}))
    _cache["last_res"] = res
    out = np.concatenate(
        [res.results[c]["out"][None] for c in range(N_CORES)], 0)
    return out.astype(np.float32).reshape(B, C, HW, HW)

